# revision 17
# baseline (speedup 1.0000x reference)
"""AttnDecoderLSTM single-step, sharded across 8 NeuronCores.

Sharding (core m of 8):
  - LSTM gate rows sharded by h-index slice hs = [128m, 128m+128): rows
    {i, g, o} x hs (forget gate dropped: c0 == 0). Each core computes
    h[hs] locally.
  - Wa rows sharded by hs: partial_v = Wa[hs,:].T @ h[hs]; AllGather #1
    carries [h_m | partial_v]; every core reconstructs full h and v.
  - encoder_outputs sequence-sharded (256 rows/core): local softmax
    stats + partial context; AllGather #2 carries [max, sum, partial_ctx].
  - Wl vocab-sharded (4000 rows/core), pre-transposed on host, streamed
    as h-half then ctx-half; logsumexp stats AllGather #3; log_softmax
    subtract on device. Host concatenates the 8 output shards.
"""

import numpy as np

try:
    import concourse.bass as bass
except ImportError:
    import sys

    sys.path.insert(0, "/opt/trn_rl_repo")
    import concourse.bass as bass

import concourse.bacc as bacc
import concourse.tile as tile
import concourse.mybir as mybir
import concourse.bass_isa as bass_isa
from concourse import bass_utils

F32 = mybir.dt.float32
AF = mybir.ActivationFunctionType
ALU = mybir.AluOpType

H = 1024
SEQ = 2048
V = 32000
NC = 8
HS = H // NC          # 128  h-slice per core
SS = SEQ // NC        # 256  seq-slice per core
VS = V // NC          # 4000 vocab-slice per core
NZC = 25              # contraction chunks for gates: 3072 inputs + bias pad -> 25*128
NT = 8                # logits tiles per core
TW = VS // NT         # 500  logits tile width
P1 = 1152             # AG#1 payload floats: 128 h + 1024 v
P2 = 1032             # AG#2 payload floats: m, s, 1024 ctx, 6 pad
P3 = 8                # AG#3 payload floats: M2, S2, 6 pad

_cache = {}


def _build():
    """Build + compile the 8-core SPMD Bass program (cached per process)."""
    if "nc" in _cache:
        return _cache["nc"]

    nc = bacc.Bacc("TRN2", target_bir_lowering=False, debug=False,
                   enable_asserts=True, num_devices=NC)

    # device inputs (per-core data differs, same shapes)
    d_zc = nc.dram_tensor("zc", [128, NZC], F32, kind="ExternalInput")
    d_gw = nc.dram_tensor("gw", [NZC, 128, 384], F32, kind="ExternalInput")
    d_wa = nc.dram_tensor("wa", [128, H], F32, kind="ExternalInput")
    d_et = nc.dram_tensor("encT", [NC, 128, SS], F32, kind="ExternalInput")
    d_en = nc.dram_tensor("encN", [2, 128, H], F32, kind="ExternalInput")
    d_wlh = nc.dram_tensor("wlh", [NT, 8, 128, TW], F32, kind="ExternalInput")
    d_wlc = nc.dram_tensor("wlc", [NT, 8, 128, TW], F32, kind="ExternalInput")
    # bias/output in [k(4), j(2), TW] layout: logits tile t lives at PSUM/SBUF
    # partition 32*(t%4), column block t//4  (t = j*4 + k)
    d_bl = nc.dram_tensor("bl", [4, 2, TW], F32, kind="ExternalInput")
    d_out = nc.dram_tensor("out", [4, 2, TW], F32, kind="ExternalOutput")

    rg = [list(range(NC))]

    with tile.TileContext(nc) as tc:
        with (
            tc.tile_pool(name="wlp", bufs=8) as wlp,
            tc.tile_pool(name="wgt", bufs=1) as wgt,
            tc.tile_pool(name="sml", bufs=1) as sml,
            tc.tile_pool(name="ps", bufs=1, space="PSUM") as ps,
            tc.tile_pool(name="psl", bufs=1, space="PSUM") as psl,
            tc.tile_pool(name="dram", bufs=1, space="DRAM") as dram,
        ):
            # ---- stage 0: weight streams (sync queue, in consumption order)
            # gate weights split in 4 quarters sharing the wl pool slots
            gw_tiles = []
            gw_split = [(0, 7), (7, 13), (13, 19), (19, NZC)]
            for qi, (c0, c1) in enumerate(gw_split):
                t_gwq = wlp.tile([128, c1 - c0, 384], F32, tag="wl",
                                 name=f"t_gw{qi}")
                nc.sync.dma_start(
                    t_gwq[:], d_gw.ap()[c0:c1].rearrange("c p j -> p c j"))
                gw_tiles.append((c0, c1, t_gwq))
            t_wa = wgt.tile([128, H], F32, tag="wa")
            nc.sync.dma_start(t_wa[:], d_wa.ap())
            t_et = wgt.tile([128, NC, SS], F32, tag="encT")
            nc.sync.dma_start(t_et[:], d_et.ap().rearrange("c p s -> p c s"))
            t_en = wgt.tile([128, 2, H], F32, tag="encN")
            nc.sync.dma_start(t_en[:], d_en.ap().rearrange("c p h -> p c h"))

            # small, latency-critical loads on the scalar (ACT) queue
            t_zc = sml.tile([128, NZC], F32, tag="zc")
            nc.scalar.dma_start(t_zc[:], d_zc.ap())
            t_bl = sml.tile([128, 2, TW], F32, tag="bl")
            for k in range(4):
                nc.scalar.dma_start(t_bl[32 * k:32 * k + 1, :, :], d_bl.ap()[k])
            t_one = sml.tile([8, 1], F32, tag="one")
            nc.vector.memset(t_one[:], 1.0)
            t_id1 = sml.tile([1, 1], F32, tag="id1")
            nc.vector.memset(t_id1[:], 1.0)

            # ---- stage 1: gates = G @ z (+bias folded in), i/g/o only
            p_g = ps.tile([1, 384], F32, tag="acc")
            for c0, c1, t_gwq in gw_tiles:
                for c in range(c0, c1):
                    nc.tensor.matmul(p_g[:], lhsT=t_zc[:, c:c + 1],
                                     rhs=t_gwq[:, c - c0, :],
                                     start=(c == 0), stop=(c == NZC - 1))

            # LSTM elementwise: h = sig(o) * tanh(sig(i) * tanh(g))
            t_si = sml.tile([1, 128], F32, tag="si")
            nc.scalar.activation(t_si[:], p_g[0:1, 0:128], AF.Sigmoid)
            t_tg = sml.tile([1, 128], F32, tag="tg")
            nc.scalar.activation(t_tg[:], p_g[0:1, 128:256], AF.Tanh)
            t_so = sml.tile([1, 128], F32, tag="so")
            nc.scalar.activation(t_so[:], p_g[0:1, 256:384], AF.Sigmoid)
            t_c = sml.tile([1, 128], F32, tag="c")
            nc.vector.tensor_mul(t_c[:], t_si[:], t_tg[:])
            t_tc = sml.tile([1, 128], F32, tag="tc")
            nc.scalar.activation(t_tc[:], t_c[:], AF.Tanh)
            t_h = sml.tile([1, 128], F32, tag="h")
            nc.vector.tensor_mul(t_h[:], t_so[:], t_tc[:])

            # h row -> column via PE transpose
            p_hT = ps.tile([128, 1], F32, tag="col")
            nc.tensor.transpose(p_hT[:], t_h[:], t_id1[:])
            t_hc = sml.tile([128, 1], F32, tag="hc")
            nc.vector.tensor_copy(t_hc[:], p_hT[:])

            # partial_v[1, H] = h_col.T @ Wa[hs, :]
            p_v = ps.tile([1, H], F32, tag="acc")
            for half in range(2):
                sl = slice(half * 512, half * 512 + 512)
                nc.tensor.matmul(p_v[0:1, sl], lhsT=t_hc[:], rhs=t_wa[:, sl],
                                 start=True, stop=True)
            t_vp = sml.tile([1, H], F32, tag="vp")
            nc.vector.tensor_copy(t_vp[:], p_v[:])

            # ---- AG#1: [h_m(128) | partial_v(1024)]
            b1i = dram.tile([P1, 1], F32, tag="b1i")
            b1o = dram.tile([NC, P1], F32, addr_space="Shared", tag="b1o")
            nc.scalar.dma_start(b1i[0:128, 0:1], t_hc[:])
            nc.scalar.dma_start(b1i[128:P1, 0:1].rearrange("p one -> one p"), t_vp[:])
            nc.gpsimd.collective_compute("AllGather", ALU.bypass, replica_groups=rg,
                                         ins=[b1i[:].opt()], outs=[b1o[:].opt()])

            # gather back: h columns [128, 8] and v partials [8, 1024]
            t_hall = sml.tile([128, NC], F32, tag="hall")
            nc.scalar.dma_start(t_hall[:], b1o[:, 0:128].rearrange("r p -> p r"))
            t_vg = sml.tile([NC, H], F32, tag="vg")
            nc.scalar.dma_start(t_vg[:], b1o[:, 128:P1])

            # v columns [128, 8]: column hc = sum_r vg[r, hc*128 : hc*128+128]
            p_vc = ps.tile([128, NC], F32, tag="col")
            for hc in range(NC):
                nc.tensor.matmul(p_vc[:, hc:hc + 1],
                                 lhsT=t_vg[:, hc * 128:(hc + 1) * 128],
                                 rhs=t_one[:], start=True, stop=True)
            t_vc = sml.tile([128, NC], F32, tag="vc")
            nc.vector.tensor_copy(t_vc[:], p_vc[:])

            # ---- stage 2: attention on the local seq shard
            p_e = ps.tile([1, SS], F32, tag="acc")
            for hc in range(NC):
                nc.tensor.matmul(p_e[:], lhsT=t_vc[:, hc:hc + 1], rhs=t_et[:, hc, :],
                                 start=(hc == 0), stop=(hc == NC - 1))
            t_negm = sml.tile([1, 1], F32, tag="negm")
            nc.vector.reduce_max(t_negm[:], p_e[:], axis=mybir.AxisListType.X,
                                 negate=True)
            t_mx = sml.tile([1, 1], F32, tag="mx")
            nc.vector.tensor_scalar_mul(t_mx[:], t_negm[:], -1.0)
            t_p = sml.tile([1, SS], F32, tag="p")
            t_s = sml.tile([1, 1], F32, tag="s")
            nc.scalar.activation(t_p[:], p_e[:], AF.Exp, bias=t_negm[:],
                                 accum_out=t_s[:])
            # attn weights row -> columns [128, 2]
            t_pc = sml.tile([128, 2], F32, tag="pc")
            for sc in range(2):
                p_pT = ps.tile([128, 1], F32, tag="col")
                nc.tensor.transpose(p_pT[:], t_p[0:1, sc * 128:(sc + 1) * 128],
                                    t_id1[:])
                nc.vector.tensor_copy(t_pc[:, sc:sc + 1], p_pT[:])
            # partial ctx [1, H] = sum_sc p_col_sc.T @ encN[sc]
            p_cx = ps.tile([1, H], F32, tag="acc")
            for half in range(2):
                sl = slice(half * 512, half * 512 + 512)
                for sc in range(2):
                    nc.tensor.matmul(p_cx[0:1, sl], lhsT=t_pc[:, sc:sc + 1],
                                     rhs=t_en[:, sc, sl],
                                     start=(sc == 0), stop=(sc == 1))
            t_cx = sml.tile([1, H], F32, tag="cx")
            nc.vector.tensor_copy(t_cx[:], p_cx[:])

            # ---- AG#2: [m(1) | s(1) | partial_ctx(1024) | pad]
            b2i = dram.tile([P2, 1], F32, tag="b2i")
            b2o = dram.tile([NC, P2], F32, addr_space="Shared", tag="b2o")
            nc.scalar.dma_start(b2i[0:1, 0:1], t_mx[:])
            nc.scalar.dma_start(b2i[1:2, 0:1], t_s[:])
            nc.scalar.dma_start(b2i[2:2 + H, 0:1].rearrange("p one -> one p"), t_cx[:])
            nc.gpsimd.collective_compute("AllGather", ALU.bypass, replica_groups=rg,
                                         ins=[b2i[:].opt()], outs=[b2o[:].opt()])

            t_nm = sml.tile([NC, 1], F32, tag="nm")
            nc.scalar.dma_start(t_nm[:], b2o[:, 0:1])
            t_ss = sml.tile([NC, 1], F32, tag="ss")
            nc.scalar.dma_start(t_ss[:], b2o[:, 1:2])
            t_cg = sml.tile([NC, H], F32, tag="cg")
            nc.scalar.dma_start(t_cg[:], b2o[:, 2:2 + H])

            # softmax combine across cores (column layout [8,1])
            t_M = sml.tile([NC, 1], F32, tag="M")
            nc.gpsimd.partition_all_reduce(t_M[:], t_nm[:], channels=NC,
                                           reduce_op=bass_isa.ReduceOp.max)
            # a_r = exp(m_r - M)
            t_a = sml.tile([NC, 1], F32, tag="a")
            nc.vector.tensor_sub(t_a[:], t_nm[:], t_M[:])
            nc.scalar.activation(t_a[:], t_a[:], AF.Exp)
            t_w = sml.tile([NC, 1], F32, tag="w")
            nc.vector.tensor_mul(t_w[:], t_a[:], t_ss[:])
            t_S = sml.tile([NC, 1], F32, tag="S")
            nc.gpsimd.partition_all_reduce(t_S[:], t_w[:], channels=NC,
                                           reduce_op=bass_isa.ReduceOp.add)
            t_rS = sml.tile([NC, 1], F32, tag="rS")
            nc.vector.reciprocal(t_rS[:], t_S[:])
            t_an = sml.tile([NC, 1], F32, tag="an")
            nc.vector.tensor_mul(t_an[:], t_a[:], t_rS[:])

            # ctx columns [128, 8]: col hc = sum_r an_r * cg[r, hc*128:...]
            p_cc = ps.tile([128, NC], F32, tag="col")
            for hc in range(NC):
                nc.tensor.matmul(p_cc[:, hc:hc + 1],
                                 lhsT=t_cg[:, hc * 128:(hc + 1) * 128],
                                 rhs=t_an[:], start=True, stop=True)
            t_cc = sml.tile([128, NC], F32, tag="cc")
            nc.vector.tensor_copy(t_cc[:], p_cc[:])

            # ---- stage 3: logits = Wl @ [h; ctx] + bl, streamed h-half then c-half
            # 8 tiles of 500 logits; tile t = j*4+k lives at partition 32k,
            # column block j; PSUM: 2 banks [128, TW].
            p_l = [psl.tile([128, TW], F32, tag=f"lg{i}", name=f"p_l{i}")
                   for i in range(2)]

            def lslot(t):
                return p_l[t // 4][(t % 4) * 32:(t % 4) * 32 + 1, :]

            # biased logits + per-tile stats, all kept at partition 32k
            t_lsb = sml.tile([128, 2, TW], F32, tag="lsb")
            nc.vector.memset(t_lsb[:], 0.0)
            t_p2 = sml.tile([128, 2, TW], F32, tag="p2")
            t_st = sml.tile([128, 2, 2], F32, tag="st")  # [.., j, (negmax, sum)]

            for phase, d_w, yc in ((0, d_wlh, t_hall), (1, d_wlc, t_cc)):
                for t in range(NT):
                    t_wl = wlp.tile([128, 8, TW], F32, tag="wl")
                    nc.sync.dma_start(t_wl[:], d_w.ap()[t].rearrange("c p r -> p c r"))
                    tp = (0, (t % 4) * 32)
                    for c in range(8):
                        nc.tensor.matmul(lslot(t), lhsT=yc[:, c:c + 1],
                                         rhs=t_wl[:, c, :],
                                         start=(phase == 0 and c == 0),
                                         stop=(phase == 1 and c == 7),
                                         tile_position=tp)
                    if phase == 1:
                        k, j = t % 4, t // 4
                        r = slice(32 * k, 32 * k + 1)
                        nc.vector.tensor_add(t_lsb[r, j, :], lslot(t), t_bl[r, j, :])
                        nc.vector.reduce_max(t_st[r, j, 0:1], t_lsb[r, j, :],
                                             axis=mybir.AxisListType.X, negate=True)
                        nc.scalar.activation(t_p2[r, j, :], t_lsb[r, j, :], AF.Exp,
                                             bias=t_st[r, j, 0:1],
                                             accum_out=t_st[r, j, 1:2])

            # ---- AG#3: all 16 per-tile stats [k(4), j(2), (negmax, sum)]
            b3i = dram.tile([4, 2, 2], F32, tag="b3i")
            b3o = dram.tile([NC, 16], F32, addr_space="Shared", tag="b3o")
            for k in range(4):
                nc.scalar.dma_start(b3i[k], t_st[32 * k:32 * k + 1, :, :])
            nc.gpsimd.collective_compute("AllGather", ALU.bypass, replica_groups=rg,
                                         ins=[b3i[:].opt()], outs=[b3o[:].opt()])
            t_g3 = sml.tile([NC, 8, 2], F32, tag="g3")
            nc.scalar.dma_start(t_g3[:], b3o[:].rearrange("p (e two) -> p e two",
                                                          two=2))

            # global LSE = Mg + log(sum exp(m - Mg) * s) over all 64 tile stats
            # g3[:, :, 0] holds NEGATED maxes
            t_nmrow = sml.tile([NC, 1], F32, tag="nmrow")
            nc.vector.tensor_reduce(t_nmrow[:], t_g3[:, :, 0:1],
                                    axis=mybir.AxisListType.XY, op=ALU.min)
            t_Mrow = sml.tile([NC, 1], F32, tag="Mrow")
            nc.vector.tensor_scalar_mul(t_Mrow[:], t_nmrow[:], -1.0)
            t_Mg = sml.tile([NC, 1], F32, tag="Mg")
            nc.gpsimd.partition_all_reduce(t_Mg[:], t_Mrow[:], channels=NC,
                                           reduce_op=bass_isa.ReduceOp.max)
            t_negMg = sml.tile([NC, 1], F32, tag="negMg")
            nc.vector.tensor_scalar_mul(t_negMg[:], t_Mg[:], -1.0)
            # a = exp(m - Mg) = exp(-(nm) - Mg) = exp(nm * -1 + (-Mg))
            t_a3 = sml.tile([NC, 8], F32, tag="a3")
            nc.scalar.activation(t_a3[:], t_g3[:, :, 0:1].rearrange("p e one -> p (e one)"),
                                 AF.Exp, bias=t_negMg[:], scale=-1.0)
            nc.vector.tensor_mul(t_a3[:], t_a3[:],
                                 t_g3[:, :, 1:2].rearrange("p e one -> p (e one)"))
            t_Srow = sml.tile([NC, 1], F32, tag="Srow")
            nc.vector.reduce_sum(t_Srow[:], t_a3[:], axis=mybir.AxisListType.X)
            t_Sg = sml.tile([NC, 1], F32, tag="Sg")
            nc.gpsimd.partition_all_reduce(t_Sg[:], t_Srow[:], channels=NC,
                                           reduce_op=bass_isa.ReduceOp.add)
            t_lse = sml.tile([NC, 1], F32, tag="lse")
            nc.scalar.activation(t_lse[:], t_Sg[:], AF.Ln)
            nc.vector.tensor_add(t_lse[:], t_lse[:], t_Mg[:])
            t_lse128 = sml.tile([128, 1], F32, tag="lse128")
            nc.gpsimd.partition_broadcast(t_lse128[:], t_lse[0:1, 0:1])

            # out = logits - LSE (garbage partitions included; host ignores)
            t_out = sml.tile([128, 2, TW], F32, tag="out")
            nc.vector.tensor_scalar(t_out[:], t_lsb[:], t_lse128[:], None,
                                    op0=ALU.subtract)
            for k in range(4):
                nc.sync.dma_start(d_out.ap()[k], t_out[32 * k:32 * k + 1, :, :])

    nc.compile()
    _cache["nc"] = nc
    return nc


def host_prep(word_input, last_context, last_hidden, encoder_outputs,
              emb, W_ih, W_hh, b_ih, b_hh, Wa, ba, Wl, bl):
    """Shard + lay out the full inputs into per-core device input maps."""
    f32 = np.float32
    idx = int(np.asarray(word_input).reshape(-1)[0])
    x = np.asarray(emb)[idx].astype(f32)

    z = np.concatenate([x, np.asarray(last_context, f32)[0],
                        np.asarray(last_hidden, f32)[0]])          # [3072]
    zp = np.zeros(NZC * 128, f32)
    zp[:3 * H] = z
    zp[3 * H] = 1.0                                                # bias lane
    z_cols = np.ascontiguousarray(zp.reshape(NZC, 128).T)          # [128, 25]

    W = np.concatenate([np.asarray(W_ih, f32), np.asarray(W_hh, f32)], axis=1)
    bsum = np.asarray(b_ih, f32) + np.asarray(b_hh, f32)
    enc = np.asarray(encoder_outputs, f32)
    Wl = np.asarray(Wl, f32)
    Wa = np.asarray(Wa, f32)
    bl = np.asarray(bl, f32)

    in_maps = []
    for m in range(NC):
        hs = np.arange(m * HS, (m + 1) * HS)
        rows = np.concatenate([hs, 2 * H + hs, 3 * H + hs])        # i, g, o
        Gm = W[rows]                                               # [384, 3072]
        gw = np.zeros((NZC, 128, 384), f32)
        gw.reshape(NZC * 128, 384)[:3 * H] = Gm.T
        gw[24, 0, :] = bsum[rows]

        ss = slice(m * SS, (m + 1) * SS)
        encT = np.ascontiguousarray(enc[ss].T).reshape(NC, 128, SS)
        encN = np.ascontiguousarray(enc[ss]).reshape(2, 128, H)

        vs = slice(m * VS, (m + 1) * VS)
        WhT = np.ascontiguousarray(Wl[vs, 0:H].T)                  # [1024, 4000]
        wlh = np.ascontiguousarray(
            WhT.reshape(8, 128, NT, TW).transpose(2, 0, 1, 3))     # [8,8,128,500]
        WcT = np.ascontiguousarray(Wl[vs, H:2 * H].T)
        wlc = np.ascontiguousarray(
            WcT.reshape(8, 128, NT, TW).transpose(2, 0, 1, 3))

        # bias in [k(4), j(2), TW]: tile t = j*4 + k covers bl[vs][t*TW:(t+1)*TW]
        bl4 = np.ascontiguousarray(
            bl[vs].reshape(2, 4, TW).transpose(1, 0, 2))

        in_maps.append({
            "zc": z_cols,
            "gw": gw,
            "wa": np.ascontiguousarray(Wa[hs]),                    # [128, 1024]
            "encT": encT,
            "encN": encN,
            "wlh": wlh,
            "wlc": wlc,
            "bl": bl4,
        })
    return in_maps


def kernel(**inputs):
    in_maps = host_prep(**inputs)
    nc = _build()
    res = bass_utils.run_bass_kernel_spmd(nc, in_maps, core_ids=list(range(NC)))
    # out[k, j, r] -> logits index (j*4 + k)*TW + r
    shards = [res.results[m]["out"].transpose(1, 0, 2).reshape(VS)
              for m in range(NC)]
    return np.concatenate(shards)[None, :]
